# revision 4
# baseline (speedup 1.0000x reference)
"""Trainium2 Bass kernel for the CAP loss (camera-aware proxy memory bank).

Strategy (8 NeuronCores, SPMD, raw Bass engine blocks):
  - The center bank [32000, 2048] is sharded along the center axis: 4000
    centers (= 500 labels x 8 cams, label-major) per core, pre-transposed,
    scaled by 32 and cast to fp8e4m3 on the host so each core streams a
    [2048, 4000] fp8 shard as 8 contiguous ~1MB slabs (split across two
    DMA queues: sync ring takes even chunks, vector ring odd chunks).
  - feats are replicated, normalized on the host, scaled by 64 and cast to
    fp8. The [256, 4000] similarity tile per core is computed as 2x8x8 PE
    matmuls in fp8 DoubleRow perf mode (two 128-row k-tiles per matmul,
    K=2048 accumulated in PSUM over 8 DoubleRow steps). exp is applied on
    the scalar engine straight out of PSUM with the constant scale
    1/(T*64*32) and written to SBUF as bf16.
  - Because the bank is label-major with C=8 cams, every mask in the loss
    is a static stride pattern: intra-cam denominators are per-residue
    (mod 8) sums, the same-label sums are per-8-block sums, and the
    first-50 hard-negative sum is a prefix over global columns [0,50)/
    [0,58) (core 0). All are strided vector-engine reductions over the
    bf16 exp tile - no gathers on device.
  - The PE clock (HAM gate) is warmed by 6 dummy matmuls on a zeroed tile
    that depend only on a DVE memset, so warmup overlaps the first DMAs.
  - The own-logit numerator and the tiny [256]-sized tail (log, segment
    means over labels/cams) run on the host at gather time.

Raw Bass (nc.Block) is used instead of the Tile framework: the installed
walrus rejects two raw-ISA instructions Tile's exit barrier emits
(EVENT_SEMAPHORE_RANGE_CLEAR, multi-wait DRAIN) and InstTensorTensorReduce.
"""

import numpy as np
import ml_dtypes
from contextlib import ExitStack

import concourse.bass as bass
from concourse import mybir
from concourse.bass_utils import run_bass_kernel_spmd

# problem constants (hardcoded per harness contract)
N, D, M = 256, 2048, 32000
L, C = 4000, 8
T = 0.07
LAMDA = 0.5
NCORES = 8
SHARD = M // NCORES          # 4000 centers per core
LBL_SHARD = SHARD // C       # 500 labels per core
KT = D // 128                # 16 k-tiles
DK = KT // 2                 # 8 DoubleRow k-steps (256 rows each)
NCHUNK = 8
W_FULL = 512                 # chunk width (64 whole labels, 0 mod 8)
W_LAST = SHARD - 7 * W_FULL  # 416 (52 whole labels)
CW = [W_FULL] * 7 + [W_LAST]
NPSUM = 4                    # psum bank pairs: PE runs up to 4 chunks ahead
NW = 6                       # HAM warmup matmuls (no data dependency)

AF = 64.0                    # host fp8 scale for normalized feats
AC = 32.0                    # host fp8 scale for (unit-norm) centers
S_EXP = 1.0 / (T * AF * AC)  # constant exp scale: sims = psum * S_EXP

F32 = mybir.dt.float32
BF16 = mybir.dt.bfloat16
FP8 = mybir.dt.float8e4
ADD = mybir.AluOpType.add
AX = mybir.AxisListType.X
EXP = mybir.ActivationFunctionType.Exp
DROW = mybir.MatmulPerfMode.DoubleRow

# layout of the consolidated small output [128, 2, 66] per (p, m):
#   cols 8n+r (n<8, r<8) = per-chunk camera-residue exp sums (512 = 0 mod 8,
#       so chunk-local residue == global residue; host just sums chunks)
#   cols 64:66 = prefix sums P50, P58 (host uses core 0's)
SM_W = 66


def _build_program() -> bass.Bass:
    nc = bass.Bass()
    cTa = nc.dram_tensor("cTa", [7, 128, KT, W_FULL], FP8, kind="ExternalInput")
    cTb = nc.dram_tensor("cTb", [128, KT, W_LAST], FP8, kind="ExternalInput")
    fT = nc.dram_tensor("fT", [128, KT, N], FP8, kind="ExternalInput")
    sm_out = nc.dram_tensor("SM_out", [128, 2, SM_W], F32, kind="ExternalOutput")
    bs_out = nc.dram_tensor("BS_out", [128, 2, LBL_SHARD], F32,
                            kind="ExternalOutput")

    with ExitStack() as ctx:
        e = ctx.enter_context

        ft_sb = e(nc.sbuf_tensor("ft_sb", [128, KT, N], FP8))
        slabs = [e(nc.sbuf_tensor(f"slab{j}", [128, KT, W_FULL], FP8))
                 for j in range(NCHUNK)]
        et = e(nc.sbuf_tensor("et", [128, 2, SHARD], BF16))
        bs = e(nc.sbuf_tensor("bs", [128, 2, LBL_SHARD], F32))
        small = e(nc.sbuf_tensor("small", [128, 2, SM_W], F32))
        wm = e(nc.sbuf_tensor("wm", [128, W_FULL], FP8))

        ps = [[e(nc.psum_tensor(f"ps{b}_{m}", [128, W_FULL], F32))
               for m in range(2)] for b in range(NPSUM)]

        sem_ft = e(nc.semaphore("sem_ft"))
        sem_slab = [e(nc.semaphore(f"sem_slab{j}")) for j in range(NCHUNK)]
        sem_pe = e(nc.semaphore("sem_pe"))
        sem_act = e(nc.semaphore("sem_act"))
        c_v = e(nc.semaphore("c_v"))       # DVE reduce progress
        c_ws = e(nc.semaphore("c_ws"))     # warmup tile memset done
        c_warm = e(nc.semaphore("c_warm"))
        sem_od = e(nc.semaphore("sem_od"))

        block = e(nc.Block(no_gpsimd_drain=True))

        @block.sync
        def _(sync):
            # even chunks on the sync ring; chunk 0 split so the PE can
            # start on k-tiles 0..3 while the rest is still in flight
            sync.dma_start(out=slabs[0][:, 0:4, :],
                           in_=cTa[0, :, 0:4, :]).then_inc(sem_slab[0], 16)
            sync.dma_start(out=slabs[0][:, 4:16, :],
                           in_=cTa[0, :, 4:16, :]).then_inc(sem_slab[0], 16)
            for n in (2, 4, 6):
                sync.dma_start(out=slabs[n][:, :, :],
                               in_=cTa[n]).then_inc(sem_slab[n], 16)
            # label-block sums written back as their chunks complete
            sync.wait_ge(c_v, 9)
            sync.dma_start(out=bs_out[:, :, 0:256],
                           in_=bs[:, :, 0:256]).then_inc(sem_od, 16)
            sync.wait_ge(c_v, 13)
            sync.dma_start(out=bs_out[:, :, 256:384],
                           in_=bs[:, :, 256:384]).then_inc(sem_od, 16)
            sync.wait_ge(c_v, 17)
            sync.dma_start(out=bs_out[:, :, 384:500],
                           in_=bs[:, :, 384:500]).then_inc(sem_od, 16)
            sync.wait_ge(c_v, 18)
            sync.dma_start(out=sm_out[:, :, :],
                           in_=small[:, :, :]).then_inc(sem_od, 16)
            sync.wait_ge(sem_od, 64)

        @block.tensor
        def _(tensor):
            # dummy matmuls on a zeroed tile: warms the PE clock gate (HAM)
            # while the first center slab is still in flight
            tensor.wait_ge(c_ws, 1)
            last = None
            for _w in range(NW):
                last = tensor.matmul(ps[NPSUM - 1][0][:, :], wm[:, 0:128],
                                     wm[:, :], start=True, stop=True)
            last.then_inc(c_warm, 1)
            for n in range(NCHUNK):
                b = n % NPSUM
                w = CW[n]
                if n == 0:
                    tensor.wait_ge(sem_ft, 16)       # k-tiles 0..3 of feats
                    tensor.wait_ge(sem_slab[0], 16)  # k-tiles 0..3 of slab 0
                else:
                    tensor.wait_ge(sem_slab[n], 16)
                if n >= NPSUM:
                    # psum bank pair free once ACT consumed chunk n-NPSUM
                    tensor.wait_ge(sem_act, 2 * (n - NPSUM + 1))
                if n == NPSUM - 1:
                    # warmup dummies wrote this psum bank (WAW ordering)
                    tensor.wait_ge(c_warm, 1)
                last = None
                for dk in range(DK):
                    if n == 0 and dk == 2:
                        tensor.wait_ge(sem_ft, 32)
                        tensor.wait_ge(sem_slab[0], 32)
                    for m in range(2):
                        last = tensor.matmul(
                            ps[b][m][:, 0:w],
                            ft_sb[:, 2 * dk:2 * dk + 2, m * 128:(m + 1) * 128],
                            slabs[n][:, 2 * dk:2 * dk + 2, 0:w],
                            start=(dk == 0), stop=(dk == DK - 1),
                            perf_mode=DROW)
                last.then_inc(sem_pe, 1)

        @block.scalar
        def _(scalar):
            # feats + odd chunks ride the ACT engine's own HW-DGE ring (a
            # second DMA queue), in parallel with the sync ring's even chunks
            scalar.dma_start(out=ft_sb[:, 0:4, :],
                             in_=fT[:, 0:4, :]).then_inc(sem_ft, 16)
            scalar.dma_start(out=ft_sb[:, 4:16, :],
                             in_=fT[:, 4:16, :]).then_inc(sem_ft, 16)
            for n in (1, 3, 5):
                scalar.dma_start(out=slabs[n][:, :, :],
                                 in_=cTa[n]).then_inc(sem_slab[n], 16)
            scalar.dma_start(out=slabs[7][:, :, 0:W_LAST],
                             in_=cTb[:, :, :]).then_inc(sem_slab[7], 16)
            # exp stream straight out of PSUM with a constant scale
            for n in range(NCHUNK):
                b = n % NPSUM
                w = CW[n]
                scalar.wait_ge(sem_pe, n + 1)
                for m in range(2):
                    scalar.activation(
                        out=et[:, m, n * W_FULL:n * W_FULL + w],
                        in_=ps[b][m][:, 0:w],
                        func=EXP, scale=S_EXP).then_inc(sem_act, 1)

        @block.vector
        def _(vector):
            vector.memset(wm[:, :], 0.0).then_inc(c_ws, 1)

            vcount = 0

            def v(instr):
                nonlocal vcount
                instr.then_inc(c_v, 1)
                vcount += 1

            # prefix sums over global columns [0,50)/[0,58) (host uses core
            # 0's)
            vector.wait_ge(sem_act, 2)
            v(vector.tensor_reduce(out=small[:, :, 64:65], in_=et[:, :, 0:50],
                                   axis=AX, op=ADD))
            v(vector.tensor_reduce(out=small[:, :, 65:66], in_=et[:, :, 0:58],
                                   axis=AX, op=ADD))
            # per-chunk reductions right behind each exp: label-block sums
            # and camera-residue sums (chunks are 0 mod 8 wide)
            for n in range(NCHUNK):
                w = CW[n]
                nl = w // C                                        # 64 or 52
                vector.wait_ge(sem_act, 2 * (n + 1))
                chunk = et[:, :, n * W_FULL:n * W_FULL + w]
                v(vector.tensor_reduce(
                    out=bs[:, :, 64 * n:64 * n + nl],
                    in_=chunk.rearrange("p m (l r) -> p m l r", r=C),
                    axis=AX, op=ADD))
                v(vector.tensor_reduce(
                    out=small[:, :, 8 * n:8 * n + 8],
                    in_=chunk.rearrange("p m (l r) -> p m r l", r=C),
                    axis=AX, op=ADD))
            assert vcount == 18

    return nc


_PROGRAM_CACHE: dict[str, bass.Bass] = {}


def _program() -> bass.Bass:
    if "nc" not in _PROGRAM_CACHE:
        _PROGRAM_CACHE["nc"] = _build_program()
    return _PROGRAM_CACHE["nc"]


def _make_in_maps(feats, centers):
    fp8 = ml_dtypes.float8_e4m3
    f = feats / np.linalg.norm(feats, axis=1, keepdims=True)
    fq = (f * AF).astype(fp8)                          # [256, 2048]
    fT_t = np.ascontiguousarray(fq.T)                  # [2048, 256]
    fT_t = np.ascontiguousarray(
        fT_t.reshape(KT, 128, N).transpose(1, 0, 2))   # [128, 16, 256]
    cq = (centers * AC).astype(fp8)                    # [32000, 2048] fp8

    in_maps = []
    for c in range(NCORES):
        shard = np.ascontiguousarray(
            cq[c * SHARD:(c + 1) * SHARD].T)             # [2048, 4000]
        sk = shard.reshape(KT, 128, SHARD)               # [16, 128, 4000]
        a = sk[:, :, 0:7 * W_FULL].reshape(KT, 128, 7, W_FULL)
        a = np.ascontiguousarray(a.transpose(2, 1, 0, 3))  # [7, 128, 16, 512]
        b = np.ascontiguousarray(
            sk[:, :, 7 * W_FULL:].transpose(1, 0, 2))      # [128, 16, 416]
        in_maps.append({"cTa": a, "cTb": b, "fT": fT_t})
    return in_maps


def _host_tail(results, feats, centers, labels, camids, epoch):
    n = labels.shape[0]
    # SM_out [128, 2, SM_W]: sample i lives at [i % 128, i // 128, :]
    SM = [r["SM_out"].transpose(1, 0, 2).reshape(n, SM_W) for r in results]
    # per-chunk camera-residue sums (aligned: just sum over chunks and cores)
    S = np.zeros((n, C), np.float32)
    for sm in SM:
        S += sm[:, 0:64].reshape(n, NCHUNK, C).sum(axis=1)
    denom_intra = S[np.arange(n), camids]

    owner = (labels // LBL_SHARD).astype(np.int64)
    BS = np.stack([r["BS_out"].transpose(1, 0, 2).reshape(n, LBL_SHARD)
                   for r in results])
    B = BS[owner, np.arange(n), labels % LBL_SHARD]
    p50, p58 = SM[0][:, 64], SM[0][:, 65]
    hard = np.where(labels <= 6, p58 - B, p50)
    denom_inter = B + hard

    # own-logit numerator in f32 on the host (256 x 2048 dot)
    f = feats / np.linalg.norm(feats, axis=1, keepdims=True)
    own = np.einsum("nd,nd->n", f,
                    centers[labels * C + camids]).astype(np.float32) / T

    loss_i = own - np.log(denom_intra)
    loss_j = own - np.log(denom_inter)

    cam_sums = np.zeros(C, np.float32)
    cam_cnts = np.zeros(C, np.float32)
    np.add.at(cam_sums, camids, loss_i)
    np.add.at(cam_cnts, camids, 1.0)
    loss_intra = -np.sum(
        np.where(cam_cnts > 0, cam_sums / np.maximum(cam_cnts, 1.0), 0.0),
        dtype=np.float32)

    lbl_sums = np.zeros(L, np.float32)
    lbl_cnts = np.zeros(L, np.float32)
    np.add.at(lbl_sums, labels, loss_j)
    np.add.at(lbl_cnts, labels, 1.0)
    loss_inter = -np.sum(
        np.where(lbl_cnts > 0, lbl_sums / np.maximum(lbl_cnts, 1.0), 0.0),
        dtype=np.float32)

    if int(epoch) < 5:
        return np.float32(loss_intra)
    return np.stack([loss_intra, LAMDA * loss_inter]).astype(np.float32)


def kernel(feats, centers, labels, camids, epoch):
    feats = np.ascontiguousarray(np.asarray(feats, dtype=np.float32))
    centers = np.ascontiguousarray(np.asarray(centers, dtype=np.float32))
    labels = np.asarray(labels).astype(np.int64)
    camids = np.asarray(camids).astype(np.int64)

    in_maps = _make_in_maps(feats, centers)
    res = run_bass_kernel_spmd(_program(), in_maps, list(range(NCORES))).results
    return _host_tail(res, feats, centers, labels, camids, epoch)


# revision 20
# speedup vs baseline: 1.1434x; 1.1434x over previous
"""Trainium2 Bass kernel for the CAP loss (camera-aware proxy memory bank).

Strategy (8 NeuronCores, SPMD, raw Bass engine blocks):
  - The center bank [32000, 2048] is sharded along the center axis: 4000
    centers (= 500 labels x 8 cams, label-major) per core, pre-transposed,
    scaled by 32 and cast to fp8e4m3 on the host so each core streams a
    [2048, 4000] fp8 shard as 8 contiguous ~1MB slabs (split across two
    DMA queues: sync ring takes even chunks, vector ring odd chunks).
  - feats are replicated, normalized on the host, scaled by 64 and cast to
    fp8. The [256, 4000] similarity tile per core is computed as 2x8x8 PE
    matmuls in fp8 DoubleRow perf mode (two 128-row k-tiles per matmul,
    K=2048 accumulated in PSUM over 8 DoubleRow steps). exp is applied on
    the scalar engine straight out of PSUM with the constant scale
    1/(T*64*32) and written to SBUF as bf16.
  - Because the bank is label-major with C=8 cams, every mask in the loss
    is a static stride pattern: intra-cam denominators are per-residue
    (mod 8) sums, the same-label sums are per-8-block sums, and the
    first-50 hard-negative sum is a prefix over global columns [0,50)/
    [0,58) (core 0). All are strided vector-engine reductions over the
    bf16 exp tile - no gathers on device.
  - The PE clock (HAM gate) is warmed by 6 dummy matmuls on a zeroed tile
    that depend only on a DVE memset, so warmup overlaps the first DMAs.
  - The own-logit numerator and the tiny [256]-sized tail (log, segment
    means over labels/cams) run on the host at gather time.

Raw Bass (nc.Block) is used instead of the Tile framework: the installed
walrus rejects two raw-ISA instructions Tile's exit barrier emits
(EVENT_SEMAPHORE_RANGE_CLEAR, multi-wait DRAIN) and InstTensorTensorReduce.
"""

import numpy as np
import ml_dtypes
from contextlib import ExitStack

import concourse.bass as bass
from concourse import mybir
from concourse.bass_utils import run_bass_kernel_spmd

# problem constants (hardcoded per harness contract)
N, D, M = 256, 2048, 32000
L, C = 4000, 8
T = 0.07
LAMDA = 0.5
NCORES = 8
SHARD = M // NCORES          # 4000 centers per core
LBL_SHARD = SHARD // C       # 500 labels per core
KT = D // 128                # 16 k-tiles
DK = KT // 2                 # 8 DoubleRow k-steps (256 rows each)
NCHUNK = 8
W_FULL = 512                 # chunk width (64 whole labels, 0 mod 8)
W_LAST = SHARD - 7 * W_FULL  # 416 (52 whole labels)
CW = [W_FULL] * 7 + [W_LAST]
NPSUM = 4                    # psum bank pairs: PE runs up to 4 groups ahead
NW = 14                      # HAM warmup matmuls (no data dependency)
NRED = 6                     # chunks reduced on device; chunks 6,7 ship raw
RAW0 = NRED * W_FULL         # 3072: first raw et column
RAW_W = SHARD - RAW0         # 928 raw columns (chunks 6, 7)
H_LAST = W_LAST // 2         # 208: chunk 7 runs as two half-width groups
# matmul groups: chunks 0..6 full width, then chunk 7 as two halves
GROUPS = [(n * W_FULL, W_FULL, n) for n in range(7)] + \
         [(7 * W_FULL, H_LAST, 7), (7 * W_FULL + H_LAST, H_LAST, 7)]
NGRP = len(GROUPS)           # 9

AF = 64.0                    # host fp8 scale for normalized feats
AC = 32.0                    # host fp8 scale for (unit-norm) centers
S_EXP = 1.0 / (T * AF * AC)  # constant exp scale: sims = psum * S_EXP

F32 = mybir.dt.float32
BF16 = mybir.dt.bfloat16
FP8 = mybir.dt.float8e4
ADD = mybir.AluOpType.add
AX = mybir.AxisListType.X
EXP = mybir.ActivationFunctionType.Exp
DROW = mybir.MatmulPerfMode.DoubleRow

# layout of the consolidated small output [128, 2, 50] per (p, m):
#   cols 8n+r (n<6, r<8) = per-chunk camera-residue exp sums (512 = 0 mod 8,
#       so chunk-local residue == global residue; host just sums chunks;
#       chunks 6-7 residues come from the raw ET tile on the host)
#   cols 48:50 = prefix sums P50, P58 (host uses core 0's)
SM_W = 50
BS_W = NRED * 64             # 384 label-block sums on device; rest on host


def _build_program() -> bass.Bass:
    nc = bass.Bass()
    cTa = nc.dram_tensor("cTa", [7, 128, KT, W_FULL], FP8, kind="ExternalInput")
    cTb = nc.dram_tensor("cTb", [128, KT, W_LAST], FP8, kind="ExternalInput")
    fT = nc.dram_tensor("fT", [128, KT, N], FP8, kind="ExternalInput")
    sm_out = nc.dram_tensor("SM_out", [128, 2, SM_W], F32, kind="ExternalOutput")
    bs_out = nc.dram_tensor("BS_out", [128, 2, BS_W], F32,
                            kind="ExternalOutput")
    et_out = nc.dram_tensor("ET_out", [128, 2, RAW_W], BF16,
                            kind="ExternalOutput")

    with ExitStack() as ctx:
        e = ctx.enter_context

        ft_sb = e(nc.sbuf_tensor("ft_sb", [128, KT, N], FP8))
        slabs = [e(nc.sbuf_tensor(f"slab{j}", [128, KT, W_FULL], FP8))
                 for j in range(NCHUNK)]
        et = e(nc.sbuf_tensor("et", [128, 2, SHARD], BF16))
        bs = e(nc.sbuf_tensor("bs", [128, 2, BS_W], F32))
        small = e(nc.sbuf_tensor("small", [128, 2, SM_W], F32))
        wm = e(nc.sbuf_tensor("wm", [128, W_FULL], FP8))

        ps = [[e(nc.psum_tensor(f"ps{b}_{m}", [128, W_FULL], F32))
               for m in range(2)] for b in range(NPSUM)]

        sem_ftl = e(nc.semaphore("sem_ftl"))
        sem_fth = e(nc.semaphore("sem_fth"))
        sem_slo = [e(nc.semaphore(f"sem_slo{j}")) for j in range(NCHUNK)]
        sem_shi = [e(nc.semaphore(f"sem_shi{j}")) for j in range(NCHUNK)]
        sem_pe = e(nc.semaphore("sem_pe"))
        sem_act = e(nc.semaphore("sem_act"))
        c_v = e(nc.semaphore("c_v"))       # DVE reduce progress
        c_ws = e(nc.semaphore("c_ws"))     # warmup tile memset done
        c_warm = e(nc.semaphore("c_warm"))
        sem_od = e(nc.semaphore("sem_od"))

        block = e(nc.Block(no_gpsimd_drain=True))

        @block.sync
        def _(sync):
            # low k-halves of feats + every slab on the sync ring; the high
            # halves ride the scalar ring concurrently, so each chunk's
            # first matmuls can start on half-arrived data
            sync.dma_start(out=ft_sb[:, 0:8, :],
                           in_=fT[:, 0:8, :]).then_inc(sem_ftl, 16)
            for n in range(7):
                sync.dma_start(out=slabs[n][:, 0:8, :],
                               in_=cTa[n, :, 0:8, :]).then_inc(sem_slo[n], 16)
            sync.dma_start(out=slabs[7][:, 0:8, 0:W_LAST],
                           in_=cTb[:, 0:8, :]).then_inc(sem_slo[7], 16)
            # label-block sums written back as their chunks complete
            sync.wait_ge(c_v, 10)
            sync.dma_start(out=bs_out[:, :, 0:256],
                           in_=bs[:, :, 0:256]).then_inc(sem_od, 16)
            sync.wait_ge(c_v, 13)
            sync.dma_start(out=sm_out[:, :, :],
                           in_=small[:, :, :]).then_inc(sem_od, 16)
            sync.wait_ge(c_v, 14)
            sync.dma_start(out=bs_out[:, :, 256:384],
                           in_=bs[:, :, 256:384]).then_inc(sem_od, 16)
            # chunks 6-7 exp tiles ship raw; host does their reductions
            sync.wait_ge(sem_act, 14)
            sync.dma_start(out=et_out[:, :, 0:W_FULL],
                           in_=et[:, :, RAW0:RAW0 + W_FULL]).then_inc(
                sem_od, 16)
            sync.wait_ge(sem_act, 16)
            sync.dma_start(
                out=et_out[:, :, W_FULL:W_FULL + H_LAST],
                in_=et[:, :, RAW0 + W_FULL:RAW0 + W_FULL + H_LAST]).then_inc(
                sem_od, 16)
            sync.wait_ge(sem_act, 18)
            sync.dma_start(
                out=et_out[:, :, W_FULL + H_LAST:RAW_W],
                in_=et[:, :, RAW0 + W_FULL + H_LAST:SHARD]).then_inc(
                sem_od, 16)
            sync.wait_ge(sem_od, 96)

        @block.tensor
        def _(tensor):
            # dummy matmuls on a zeroed tile: warms the PE clock gate (HAM)
            # while the first center slab is still in flight
            tensor.wait_ge(c_ws, 1)
            last = None
            for _w in range(NW):
                last = tensor.matmul(ps[NPSUM - 1][0][:, :], wm[:, 0:128],
                                     wm[:, :], start=True, stop=True)
            last.then_inc(c_warm, 1)
            for g, (off, w, sl) in enumerate(GROUPS):
                b = g % NPSUM
                coff = off - sl * W_FULL       # column offset within the slab
                if g >= NPSUM:
                    # psum bank pair free once ACT consumed group g-NPSUM
                    tensor.wait_ge(sem_act, 2 * (g - NPSUM + 1))
                if g == NPSUM - 1:
                    # warmup dummies wrote this psum bank (WAW ordering)
                    tensor.wait_ge(c_warm, 1)
                last = None
                for dk in range(DK):
                    if dk == 0 and g <= 7:
                        if g == 0:
                            tensor.wait_ge(sem_ftl, 16)
                        tensor.wait_ge(sem_slo[sl], 16)
                    if dk == DK // 2 and g <= 7:
                        if g == 0:
                            tensor.wait_ge(sem_fth, 16)
                        tensor.wait_ge(sem_shi[sl], 16)
                    for m in range(2):
                        last = tensor.matmul(
                            ps[b][m][:, 0:w],
                            ft_sb[:, 2 * dk:2 * dk + 2, m * 128:(m + 1) * 128],
                            slabs[sl][:, 2 * dk:2 * dk + 2, coff:coff + w],
                            start=(dk == 0), stop=(dk == DK - 1),
                            perf_mode=DROW)
                last.then_inc(sem_pe, 1)

        @block.scalar
        def _(scalar):
            # high k-halves ride the ACT engine's own HW-DGE ring (a second
            # DMA queue), in parallel with the sync ring's low halves
            scalar.dma_start(out=ft_sb[:, 8:16, :],
                             in_=fT[:, 8:16, :]).then_inc(sem_fth, 16)
            for n in range(7):
                scalar.dma_start(out=slabs[n][:, 8:16, :],
                                 in_=cTa[n, :, 8:16, :]).then_inc(
                    sem_shi[n], 16)
            scalar.dma_start(out=slabs[7][:, 8:16, 0:W_LAST],
                             in_=cTb[:, 8:16, :]).then_inc(sem_shi[7], 16)
            # exp stream straight out of PSUM with a constant scale
            for g, (off, w, _sl) in enumerate(GROUPS):
                b = g % NPSUM
                scalar.wait_ge(sem_pe, g + 1)
                for m in range(2):
                    scalar.activation(
                        out=et[:, m, off:off + w],
                        in_=ps[b][m][:, 0:w],
                        func=EXP, scale=S_EXP).then_inc(sem_act, 1)

        @block.vector
        def _(vector):
            vector.memset(wm[:, :], 0.0).then_inc(c_ws, 1)

            vcount = 0

            def v(instr):
                nonlocal vcount
                instr.then_inc(c_v, 1)
                vcount += 1

            # prefix sums over global columns [0,50)/[0,58) (host uses core
            # 0's)
            vector.wait_ge(sem_act, 2)
            v(vector.tensor_reduce(out=small[:, :, 48:49], in_=et[:, :, 0:50],
                                   axis=AX, op=ADD))
            v(vector.tensor_reduce(out=small[:, :, 49:50], in_=et[:, :, 0:58],
                                   axis=AX, op=ADD))
            # per-chunk reductions right behind each exp: camera-residue
            # sums then label-block sums (chunks are 0 mod 8 wide); chunks
            # 6-7 ship raw instead (they would serialize into the tail)
            for n in range(NRED):
                vector.wait_ge(sem_act, 2 * (n + 1))
                chunk = et[:, :, n * W_FULL:(n + 1) * W_FULL]
                v(vector.tensor_reduce(
                    out=small[:, :, 8 * n:8 * n + 8],
                    in_=chunk.rearrange("p m (l r) -> p m r l", r=C),
                    axis=AX, op=ADD))
                v(vector.tensor_reduce(
                    out=bs[:, :, 64 * n:64 * n + 64],
                    in_=chunk.rearrange("p m (l r) -> p m l r", r=C),
                    axis=AX, op=ADD))
            assert vcount == 14

    return nc


_PROGRAM_CACHE: dict[str, bass.Bass] = {}


def _program() -> bass.Bass:
    if "nc" not in _PROGRAM_CACHE:
        _PROGRAM_CACHE["nc"] = _build_program()
    return _PROGRAM_CACHE["nc"]


def _make_in_maps(feats, centers):
    fp8 = ml_dtypes.float8_e4m3
    f = feats / np.linalg.norm(feats, axis=1, keepdims=True)
    fq = (f * AF).astype(fp8)                          # [256, 2048]
    fT_t = np.ascontiguousarray(fq.T)                  # [2048, 256]
    fT_t = np.ascontiguousarray(
        fT_t.reshape(KT, 128, N).transpose(1, 0, 2))   # [128, 16, 256]
    cq = (centers * AC).astype(fp8)                    # [32000, 2048] fp8

    in_maps = []
    for c in range(NCORES):
        shard = np.ascontiguousarray(
            cq[c * SHARD:(c + 1) * SHARD].T)             # [2048, 4000]
        sk = shard.reshape(KT, 128, SHARD)               # [16, 128, 4000]
        a = sk[:, :, 0:7 * W_FULL].reshape(KT, 128, 7, W_FULL)
        a = np.ascontiguousarray(a.transpose(2, 1, 0, 3))  # [7, 128, 16, 512]
        b = np.ascontiguousarray(
            sk[:, :, 7 * W_FULL:].transpose(1, 0, 2))      # [128, 16, 416]
        in_maps.append({"cTa": a, "cTb": b, "fT": fT_t})
    return in_maps


def _host_tail(results, feats, centers, labels, camids, epoch):
    n = labels.shape[0]
    # SM_out [128, 2, SM_W]: sample i lives at [i % 128, i // 128, :]
    SM = [r["SM_out"].transpose(1, 0, 2).reshape(n, SM_W) for r in results]
    # chunks 6-7 [labels 384:500] arrive as the raw exp tile; reduce here
    ETR = [r["ET_out"].transpose(1, 0, 2).reshape(n, RAW_W).astype(np.float32)
           for r in results]
    # per-chunk camera-residue sums (aligned: just sum over chunks and cores)
    S = np.zeros((n, C), np.float32)
    for sm, er in zip(SM, ETR):
        S += sm[:, 0:8 * NRED].reshape(n, NRED, C).sum(axis=1)
        S += er.reshape(n, RAW_W // C, C).sum(axis=1)
    denom_intra = S[np.arange(n), camids]

    owner = (labels // LBL_SHARD).astype(np.int64)
    BS = np.stack([
        np.concatenate(
            [r["BS_out"].transpose(1, 0, 2).reshape(n, BS_W),
             er.reshape(n, RAW_W // C, C).sum(axis=2)], axis=1)
        for r, er in zip(results, ETR)])
    B = BS[owner, np.arange(n), labels % LBL_SHARD]
    p50, p58 = SM[0][:, 8 * NRED], SM[0][:, 8 * NRED + 1]
    hard = np.where(labels <= 6, p58 - B, p50)
    denom_inter = B + hard

    # own-logit numerator in f32 on the host (256 x 2048 dot)
    f = feats / np.linalg.norm(feats, axis=1, keepdims=True)
    own = np.einsum("nd,nd->n", f,
                    centers[labels * C + camids]).astype(np.float32) / T

    loss_i = own - np.log(denom_intra)
    loss_j = own - np.log(denom_inter)

    cam_sums = np.zeros(C, np.float32)
    cam_cnts = np.zeros(C, np.float32)
    np.add.at(cam_sums, camids, loss_i)
    np.add.at(cam_cnts, camids, 1.0)
    loss_intra = -np.sum(
        np.where(cam_cnts > 0, cam_sums / np.maximum(cam_cnts, 1.0), 0.0),
        dtype=np.float32)

    lbl_sums = np.zeros(L, np.float32)
    lbl_cnts = np.zeros(L, np.float32)
    np.add.at(lbl_sums, labels, loss_j)
    np.add.at(lbl_cnts, labels, 1.0)
    loss_inter = -np.sum(
        np.where(lbl_cnts > 0, lbl_sums / np.maximum(lbl_cnts, 1.0), 0.0),
        dtype=np.float32)

    if int(epoch) < 5:
        return np.float32(loss_intra)
    return np.stack([loss_intra, LAMDA * loss_inter]).astype(np.float32)


def kernel(feats, centers, labels, camids, epoch):
    feats = np.ascontiguousarray(np.asarray(feats, dtype=np.float32))
    centers = np.ascontiguousarray(np.asarray(centers, dtype=np.float32))
    labels = np.asarray(labels).astype(np.int64)
    camids = np.asarray(camids).astype(np.int64)

    in_maps = _make_in_maps(feats, centers)
    res = run_bass_kernel_spmd(_program(), in_maps, list(range(NCORES))).results
    return _host_tail(res, feats, centers, labels, camids, epoch)
